# revision 5
# baseline (speedup 1.0000x reference)
"""Multi-head attention (B=2, S=2048, D=1024, H=16, RoPE, full softmax) on
8 TRN2 NeuronCores.

Sharding: batch x head-group. Core c = 4*b + g handles batch b and heads
[4g, 4g+4). Each core computes q/k/v projections for its 4 heads, RoPE,
scores, softmax, attention, and a partial output projection against its
head-group's w_o columns. The host sums the 4 partial outputs per batch and
adds b_o.

Device layout highlights:
  - x is shipped transposed (xT [1024, 2048] bf16) so the d-contraction sits
    on partitions for both the q/k (w stationary) and v (x stationary)
    projections.
  - q/k weight rows are packed as TE/TO m-tiles (4 heads x 32 even dims,
    then odd dims) so RoPE becomes 4 fused (psum+bias)*table muls plus one
    add/sub per group, all partition-aligned.
  - scores are computed transposed (scoresT[t, s]) with head-PAIR row
    packing: kpair/qpair tiles hold two heads at partitions 0-63 / 64-127,
    so two K=64 matmuls run concurrently on disjoint array row-strips.
  - v carries an extra ones column per head: the attnT matmul's 65th output
    row accumulates the softmax denominator for free.
  - softmax skips max-subtraction (scores are pre-scaled by 1/8 via the RoPE
    tables; |scores| < ~7 so exp is safe in fp32->bf16).
  - denominators are collected, redistributed across partitions by a small
    DMA, reciprocal'd in one batched DVE op, and broadcast back via a
    ones-column matmul.
"""

import os
import sys

for _p in ("/opt/trn_rl_repo",):
    if _p not in sys.path and os.path.isdir(_p):
        sys.path.append(_p)

import numpy as np
import ml_dtypes

import concourse.bass as bass
import concourse.mybir as mybir
from concourse.tile import TileContext
from concourse.bass_utils import run_bass_kernel_spmd

F32 = mybir.dt.float32
BF16 = mybir.dt.bfloat16
NPBF16 = ml_dtypes.bfloat16

B, S, D, H = 2, 2048, 1024, 16
HD = D // H          # 64
G = 4                # heads per core
P = 128
NCORES = 8
DC = D // P          # 8 d-chunks
ST = S // P          # 16 t-tiles
SC = S // 512        # 4 s-chunks of 512


# ---------------------------------------------------------------------------
# walrus workarounds (this container's walrus rejects >1 sync wait per
# instruction; split the TileContext exit drain and hoist extra waits onto
# NoOps on the same engine queue).
# ---------------------------------------------------------------------------
def _install_tile_patch():
    import re
    import bass_rust
    from bass_rust import ScopedClock

    if getattr(TileContext, "_drain_patched", False):
        return

    def _split_drain_and_barrier(self, tick_clock, wait_clock):
        nc = self.nc
        ticks = [int(s) for s in re.findall(r"\d+", repr(tick_clock.global_clock))]
        n = len(ticks)
        for i, t in enumerate(ticks):
            if t <= 0:
                continue
            vec = [0] * n
            vec[i] = t
            drain_inst = nc.sync.drain()
            wait_clock.add_sem_waits(
                drain_inst.ins, ScopedClock({None: bass_rust.VectorClock(vec)})
            )
        nc.sync.drain()
        nc.all_engine_barrier()
        assert self.sems is not None
        popped = nc._tile_sem_poison_stack.pop()
        assert popped is self._sem_poison
        nc.clear_and_free_semaphores(list(self.sems.allocated().values()))
        nc.all_engine_barrier()

    TileContext._drain_and_barrier = _split_drain_and_barrier
    TileContext._drain_patched = True


def _fix_multiwait(nc, max_waits=1):
    from bass_rust import SyncInfo

    n_split = 0
    for fn in nc.m.functions:
        for bb in fn.blocks:
            insts = bb.instructions
            out = []
            dirty = False
            for ins in insts:
                si = ins.sync_info
                if si is not None and si.on_wait and len(si.on_wait) > max_waits:
                    waits = list(si.on_wait)
                    for i, w in enumerate(waits[:-max_waits]):
                        nop = mybir.InstNoOp(name=f"{ins.name}-mw{i}")
                        nop.engine = ins.engine
                        nop.sync_info = SyncInfo(on_wait=[w], on_update=[])
                        out.append(nop)
                    ins.sync_info = SyncInfo(
                        on_wait=waits[-max_waits:], on_update=list(si.on_update)
                    )
                    dirty = True
                    n_split += 1
                out.append(ins)
            if dirty:
                bb.instructions = out
    return n_split


# ---------------------------------------------------------------------------
# device kernel
# ---------------------------------------------------------------------------
def _build_nc():
    _install_tile_patch()
    nc = bass.Bass()

    xT = nc.declare_dram_parameter("xT", [D, S], BF16, isOutput=False)
    wqkT = nc.declare_dram_parameter("wqkT", [D, 4 * P], BF16, isOutput=False)
    bqk = nc.declare_dram_parameter("bqk", [P, 4], F32, isOutput=False)
    wvT = nc.declare_dram_parameter("wvT", [D, G * HD], BF16, isOutput=False)
    bvb = nc.declare_dram_parameter("bvb", [P, G * HD], F32, isOutput=False)
    cosq = nc.declare_dram_parameter("cosq", [P, S], BF16, isOutput=False)
    sinq = nc.declare_dram_parameter("sinq", [P, S], BF16, isOutput=False)
    cosk = nc.declare_dram_parameter("cosk", [P, S], BF16, isOutput=False)
    sink = nc.declare_dram_parameter("sink", [P, S], BF16, isOutput=False)
    woT = nc.declare_dram_parameter("woT", [G * HD, D], BF16, isOutput=False)
    out = nc.declare_dram_parameter("out", [S, D], F32, isOutput=True)

    with TileContext(nc) as tc:
        with tc.tile_pool(name="const", bufs=1) as cpool, \
             tc.tile_pool(name="work", bufs=1) as wpool:
            # ---- resident loads -------------------------------------------
            xT_sb = cpool.tile([P, DC, S], BF16)
            nc.sync.dma_start(xT_sb[:], xT[:].rearrange("(dc p) s -> p dc s", p=P))
            wqk_sb = cpool.tile([P, DC, 4 * P], BF16)
            nc.sync.dma_start(wqk_sb[:], wqkT[:].rearrange("(dc p) m -> p dc m", p=P))
            wv_sb = cpool.tile([P, DC, G * HD], BF16)
            nc.sync.dma_start(wv_sb[:], wvT[:].rearrange("(dc p) m -> p dc m", p=P))
            wo_sb = cpool.tile([P, 2, D], BF16)
            nc.sync.dma_start(wo_sb[:], woT[:].rearrange("(jc p) d -> p jc d", p=P))
            bqk_sb = cpool.tile([P, 4], F32)
            nc.sync.dma_start(bqk_sb[:], bqk[:])
            bvb_sb = cpool.tile([P, G * HD], F32)
            nc.sync.dma_start(bvb_sb[:], bvb[:])
            tabs = {}
            for nm, src in (("cosq", cosq), ("sinq", sinq),
                            ("cosk", cosk), ("sink", sink)):
                t = cpool.tile([P, S], BF16, name=f"tab_{nm}")
                nc.sync.dma_start(t[:], src[:])
                tabs[nm] = t

            # pair tiles (2 heads each at partitions 0-63 / 64-127)
            qpair = [cpool.tile([P, S], BF16, name=f"qpair{i}") for i in range(2)]
            kpair = [cpool.tile([P, S], BF16, name=f"kpair{i}") for i in range(2)]
            # v with ones column per head: [p, t_tile, 4*65]
            vext = cpool.tile([P, ST, G * 65], BF16)
            v4 = vext[:].rearrange("p t (h c) -> p t h c", c=65)
            nc.vector.memset(v4[:, :, :, 64:65], 1.0)
            # normalized attention, assembled per pair [128 j, S] for w_o
            attn_n = [cpool.tile([P, S], BF16, name=f"attn{i}") for i in range(2)]
            # per-head raw and normalized attnT staging (base partition 0)
            attn_raw = [cpool.tile([HD, S], BF16, name=f"attnraw{i}")
                        for i in range(4)]
            attn_nh = [cpool.tile([HD, S], BF16, name=f"attnnh{i}")
                       for i in range(4)]
            # denominator machinery
            den_wide = cpool.tile([65, 8 * 512], F32)   # row 64 = collected denoms
            den_stack = cpool.tile([32, 512], F32)
            nc.vector.memset(den_stack[:], 1.0)
            den_rec = cpool.tile([32, 512], F32)
            bc64 = cpool.tile([HD, 512], F32)

            # ---- phase 1: qkv projection + rope ---------------------------
            with tc.tile_pool(name="ps_qk", bufs=6, space="PSUM") as ps_qkp, \
                 tc.tile_pool(name="ps_v", bufs=2, space="PSUM") as ps_vp, \
                 tc.tile_pool(name="rope_t", bufs=8) as rtmp:
                for sc in range(SC):
                    ssl = slice(sc * 512, (sc + 1) * 512)
                    ps_m = []
                    for m in range(4):
                        ps = ps_qkp.tile([P, 512], F32, tag="qk")
                        for dc in range(DC):
                            nc.tensor.matmul(
                                ps[:],
                                wqk_sb[:, dc, m * P:(m + 1) * P],
                                xT_sb[:, dc, ssl],
                                start=(dc == 0), stop=(dc == DC - 1),
                            )
                        ps_m.append(ps)
                    # rope: group 0 = q (m-tiles 0 TE / 1 TO), group 1 = k
                    for grp in range(2):
                        psTE, psTO = ps_m[2 * grp], ps_m[2 * grp + 1]
                        bTE = bqk_sb[:, 2 * grp:2 * grp + 1]
                        bTO = bqk_sb[:, 2 * grp + 1:2 * grp + 2]
                        cosT = tabs["cosq" if grp == 0 else "cosk"]
                        sinT = tabs["sinq" if grp == 0 else "sink"]
                        t1 = rtmp.tile([P, 512], BF16, tag="t1")
                        t2 = rtmp.tile([P, 512], BF16, tag="t2")
                        t3 = rtmp.tile([P, 512], BF16, tag="t3")
                        t4 = rtmp.tile([P, 512], BF16, tag="t4")
                        add, mult = mybir.AluOpType.add, mybir.AluOpType.mult
                        nc.vector.scalar_tensor_tensor(
                            t1[:], psTE[:], bTE, cosT[:, ssl], op0=add, op1=mult)
                        nc.vector.scalar_tensor_tensor(
                            t2[:], psTO[:], bTO, sinT[:, ssl], op0=add, op1=mult)
                        nc.vector.scalar_tensor_tensor(
                            t3[:], psTE[:], bTE, sinT[:, ssl], op0=add, op1=mult)
                        nc.vector.scalar_tensor_tensor(
                            t4[:], psTO[:], bTO, cosT[:, ssl], op0=add, op1=mult)
                        rotE = rtmp.tile([P, 512], BF16, tag="rotE")
                        rotO = rtmp.tile([P, 512], BF16, tag="rotO")
                        nc.vector.tensor_sub(rotE[:], t1[:], t2[:])
                        nc.vector.tensor_add(rotO[:], t3[:], t4[:])
                        # assemble pair tiles (te'/to' interleaved per head)
                        dst = qpair if grp == 0 else kpair
                        for pr in range(2):
                            for half, rot in ((0, rotE), (1, rotO)):
                                for hh in range(2):
                                    src_lo = (2 * pr + hh) * 32
                                    dst_lo = hh * 64 + half * 32
                                    nc.sync.dma_start(
                                        dst[pr][dst_lo:dst_lo + 32, ssl],
                                        rot[src_lo:src_lo + 32, :],
                                    )
                    # v projection for this s-chunk's 4 t-tiles
                    for tl in range(4):
                        tt = sc * 4 + tl
                        psv = ps_vp.tile([P, G * HD], F32, tag="v")
                        for dc in range(DC):
                            nc.tensor.matmul(
                                psv[:],
                                xT_sb[:, dc, tt * P:(tt + 1) * P],
                                wv_sb[:, dc, :],
                                start=(dc == 0), stop=(dc == DC - 1),
                            )
                        for h in range(G):
                            nc.vector.tensor_add(
                                vext[:, tt, h * 65:h * 65 + 64],
                                psv[:, h * HD:(h + 1) * HD],
                                bvb_sb[:, h * HD:(h + 1) * HD],
                            )

            # ---- phase 2: attention per head-pair -------------------------
            with tc.tile_pool(name="ps_s", bufs=2, space="PSUM") as ps_sp, \
                 tc.tile_pool(name="ps_a", bufs=3, space="PSUM") as ps_ap, \
                 tc.tile_pool(name="p_sb", bufs=4) as ppool:
                for pr in range(2):
                    for sc in range(SC):
                        ssl = slice(sc * 512, (sc + 1) * 512)
                        psA = ps_ap.tile([65, 512], F32, tag="at")
                        psB = ps_ap.tile([65, 512], F32, tag="at")
                        for tt in range(ST):
                            pss = ps_sp.tile([P, 1024], F32, tag="sc")
                            nc.tensor.matmul(
                                pss[:, 0:512],
                                kpair[pr][0:64, tt * P:(tt + 1) * P],
                                qpair[pr][0:64, ssl],
                                start=True, stop=True)
                            nc.tensor.matmul(
                                pss[:, 512:1024],
                                kpair[pr][64:128, tt * P:(tt + 1) * P],
                                qpair[pr][64:128, ssl],
                                start=True, stop=True)
                            p_sb = ppool.tile([P, 1024], BF16, tag="p")
                            nc.scalar.activation(
                                p_sb[:], pss[:], mybir.ActivationFunctionType.Exp)
                            hA, hB = 2 * pr, 2 * pr + 1
                            nc.tensor.matmul(
                                psA[:],
                                vext[:, tt, hA * 65:hA * 65 + 65],
                                p_sb[:, 0:512],
                                start=(tt == 0), stop=(tt == ST - 1))
                            nc.tensor.matmul(
                                psB[:],
                                vext[:, tt, hB * 65:hB * 65 + 65],
                                p_sb[:, 512:1024],
                                start=(tt == 0), stop=(tt == ST - 1))
                        for hh, psX in ((0, psA), (1, psB)):
                            idx = hh * 4 + sc
                            # stage raw attnT and collect denominator
                            nc.vector.tensor_copy(
                                attn_raw[2 * pr + hh][:, ssl], psX[0:64, :])
                            nc.vector.tensor_copy(
                                den_wide[64:65, idx * 512:(idx + 1) * 512],
                                psX[64:65, :])
                    # redistribute denominators to 8 partitions, recip once
                    for idx in range(8):
                        nc.sync.dma_start(
                            den_stack[idx:idx + 1, :],
                            den_wide[64:65, idx * 512:(idx + 1) * 512])
                    nc.vector.reciprocal(den_rec[:], den_stack[:])
                    # normalize per head: attn_nh = attn_raw * bcast(recip),
                    # then DMA-assemble the pair tile for the w_o phase
                    for hh in range(2):
                        h = 2 * pr + hh
                        for sc in range(SC):
                            idx = hh * 4 + sc
                            ssl = slice(sc * 512, (sc + 1) * 512)
                            nc.vector.stream_shuffle(
                                bc64[0:32, :], den_rec[0:32, :], mask=[idx] * 32)
                            nc.vector.stream_shuffle(
                                bc64[32:64, :], den_rec[0:32, :], mask=[idx] * 32)
                            nc.vector.tensor_mul(
                                attn_nh[h][:, ssl], attn_raw[h][:, ssl], bc64[:])
                        nc.sync.dma_start(
                            attn_n[pr][hh * 64:(hh + 1) * 64, :], attn_nh[h][:])

            # ---- phase 3: output projection -------------------------------
            with tc.tile_pool(name="ps_o", bufs=2, space="PSUM") as ps_op, \
                 tc.tile_pool(name="o_sb", bufs=3) as opool:
                for st in range(ST):
                    for half in range(2):
                        pso = ps_op.tile([P, 512], F32, tag="o")
                        for jc in range(2):
                            nc.tensor.matmul(
                                pso[:],
                                attn_n[jc][:, st * P:(st + 1) * P],
                                wo_sb[:, jc, half * 512:(half + 1) * 512],
                                start=(jc == 0), stop=(jc == 1))
                        osb = opool.tile([P, 512], F32, tag="ot")
                        nc.vector.tensor_copy(osb[:], pso[:])
                        nc.sync.dma_start(
                            out[st * P:(st + 1) * P, half * 512:(half + 1) * 512],
                            osb[:])

    _fix_multiwait(nc)
    return nc


_NC_CACHE = None


def _get_nc():
    global _NC_CACHE
    if _NC_CACHE is None:
        _NC_CACHE = _build_nc()
    return _NC_CACHE


# ---------------------------------------------------------------------------
# host-side sharding
# ---------------------------------------------------------------------------
def _deint(rows):
    """rows [64, ...] -> [even dims (32); odd dims (32)]"""
    return np.concatenate([rows[0::2], rows[1::2]], axis=0)


def _shard_inputs(input, rotations, w_qkv, b_qkv, w_o, b_o):
    x = np.asarray(input, np.float32)
    rot = np.asarray(rotations, np.float32)
    w_qkv = np.asarray(w_qkv, np.float32)
    b_qkv = np.asarray(b_qkv, np.float32)
    w_o = np.asarray(w_o, np.float32)

    cos = rot[:, :, 0].T.copy()   # [32, S]
    sin = rot[:, :, 1].T.copy()
    cos4 = np.tile(cos, (4, 1))   # [128, S]
    sin4 = np.tile(sin, (4, 1))
    cosq = (cos4 / 8.0).astype(NPBF16)
    sinq = (sin4 / 8.0).astype(NPBF16)
    cosk = cos4.astype(NPBF16)
    sink = sin4.astype(NPBF16)

    in_maps = []
    for c in range(NCORES):
        b, g = divmod(c, 4)
        heads = [4 * g + i for i in range(G)]
        xT = np.ascontiguousarray(x[b].T).astype(NPBF16)          # [D, S]

        # q/k m-tiles: TE then TO, 4 heads x 32 rows each, for q then k
        q_te, q_to, k_te, k_to, bq_te, bq_to, bk_te, bk_to = \
            [], [], [], [], [], [], [], []
        for h in heads:
            qw = _deint(w_qkv[h * HD:(h + 1) * HD])
            kw = _deint(w_qkv[D + h * HD:D + (h + 1) * HD])
            qb = _deint(b_qkv[h * HD:(h + 1) * HD])
            kb = _deint(b_qkv[D + h * HD:D + (h + 1) * HD])
            q_te.append(qw[:32]); q_to.append(qw[32:])
            k_te.append(kw[:32]); k_to.append(kw[32:])
            bq_te.append(qb[:32]); bq_to.append(qb[32:])
            bk_te.append(kb[:32]); bk_to.append(kb[32:])
        wqk = np.concatenate(
            [np.concatenate(blk, axis=0) for blk in (q_te, q_to, k_te, k_to)],
            axis=0)                                                # [512, D]
        wqkT = np.ascontiguousarray(wqk.T).astype(NPBF16)          # [D, 512]
        bqk = np.stack(
            [np.concatenate(blk) for blk in (bq_te, bq_to, bk_te, bk_to)],
            axis=1).astype(np.float32)                             # [128, 4]

        wv = np.concatenate(
            [w_qkv[2 * D + h * HD:2 * D + (h + 1) * HD] for h in heads], axis=0)
        wvT = np.ascontiguousarray(wv.T).astype(NPBF16)            # [D, 256]
        bv = np.concatenate(
            [b_qkv[2 * D + h * HD:2 * D + (h + 1) * HD] for h in heads])
        bvb = np.tile(bv[None, :], (P, 1)).astype(np.float32)      # [128, 256]

        wo = w_o[:, g * G * HD:(g + 1) * G * HD]                   # [D, 256]
        woT = np.ascontiguousarray(wo.T).astype(NPBF16)            # [256, D]

        in_maps.append({
            "xT": xT, "wqkT": wqkT, "bqk": bqk, "wvT": wvT, "bvb": bvb,
            "cosq": cosq, "sinq": sinq, "cosk": cosk, "sink": sink,
            "woT": woT,
        })
    return in_maps


def _run(inputs, trace=False):
    nc = _get_nc()
    in_maps = _shard_inputs(**inputs)
    res = run_bass_kernel_spmd(
        nc, in_maps, core_ids=list(range(NCORES)), trace=trace)
    b_o = np.asarray(inputs["b_o"], np.float32)
    out = np.zeros((B, S, D), np.float32)
    for c in range(NCORES):
        out[c // 4] += res.results[c]["out"]
    out += b_o[None, None, :]
    return out, res


def kernel(**inputs):
    out, _ = _run(inputs, trace=False)
    return out
